# revision 1
# baseline (speedup 1.0000x reference)
# ContextQueryAttention (BiDAF-style) Trainium2 Bass/Tile kernel.
#
# Full-input contract: kernel(**inputs) takes the full arrays
#   context [32, 2048, 128] f32, query [32, 128, 128] f32,
#   w [384] f32, query_mask [32, 128] i32
# and returns out [32, 2048, 512] f32.
#
# Sharding: batch B=32 split 4-per-core across 8 NeuronCores (pure data
# parallel, no collectives).
#
# Math (per batch, C=2048, Q=128, D=128):
#   S[c,q] = ctx[c]@w1 + query[q]@w2 + (ctx[c]*w3)@query[q]
#          = alpha[c] + beta[q] + G[c,q]
#   a = softmax_q(S + maskadd);  c2q = a @ query
#   m[c] = max_q(S + maskadd);   b = softmax_c(m); q2c = b @ ctx
#   out = [ctx | c2q | ctx*c2q | ctx*q2c]
#
# Tricks used:
#  * alpha[c] cancels in softmax_q -> row softmax runs on T = G + beta'
#    (beta' = beta + mask_add), computed as exp(T) with beta' fused into
#    the ACT exp bias (per-partition bias in the [q, c] layout).
#  * |S| is O(5) for these inputs, so exp() without max-subtraction is
#    exact to fp32 roundoff.
#  * E^T = exp(T^T) [q, c] feeds the c2q matmul as stationary operand;
#    a ones-column in the moving operand emits the softmax denominator
#    Z[c] in the same matmul.
#  * exp is monotone, so max_q T = via max_q E: PE-transpose each E^T
#    tile (cheap transpose-mode matmul sharing the c2q stationary) and
#    row-max it; e_m = exp(alpha) * max_q E. No S materialization in
#    [c, q] layout at all.
#  * ctx is DMA'd straight into the ctx column blocks of four [128, 2048]
#    output assembly tiles (4 c-tiles each); one contiguous 1 MiB store
#    per assembly tile.
#
# PSUM discipline: a start=True matmul into a PSUM bank invalidates any
# in-flight accumulation chain in that bank, so every accumulation chain
# (u) owns its bank, and multi-writer banks only ever hold single-matmul
# values that are read, never accumulated onto.

import numpy as np

C = 2048
Q = 128
D = 128
B_TOTAL = 32
N_CORES = 8
B_LOCAL = B_TOTAL // N_CORES  # 4
N_CT = C // 128  # 16 c-tiles per batch
N_G = 4  # assembly groups per batch (4 c-tiles each)

_compiled = None


def _build():
    import concourse.bacc as bacc
    import concourse.tile as tile
    import concourse.mybir as mybir
    from concourse import masks

    f32 = mybir.dt.float32
    i32 = mybir.dt.int32

    nc = bacc.Bacc(
        "TRN2",
        target_bir_lowering=False,
        debug=False,
        num_devices=N_CORES,
    )

    ctx_d = nc.dram_tensor("context", [B_LOCAL, C, D], f32, kind="ExternalInput").ap()
    qry_d = nc.dram_tensor("query", [B_LOCAL, Q, D], f32, kind="ExternalInput").ap()
    w_d = nc.dram_tensor("w", [3 * D], f32, kind="ExternalInput").ap()
    msk_d = nc.dram_tensor("query_mask", [B_LOCAL, Q], i32, kind="ExternalInput").ap()
    out_d = nc.dram_tensor("out", [B_LOCAL, C, 4 * D], f32, kind="ExternalOutput").ap()

    with tile.TileContext(nc) as tc:
        _kernel_body(tc, out_d, ctx_d, qry_d, w_d, msk_d, mybir, masks)

    nc.compile()
    return nc


def _kernel_body(tc, out_d, ctx_d, qry_d, w_d, msk_d, mybir, masks):
    from contextlib import ExitStack

    nc = tc.nc
    f32 = mybir.dt.float32
    AFT = mybir.ActivationFunctionType
    Alu = mybir.AluOpType

    es = ExitStack()
    with es:
        # ---- pools ----
        consts = es.enter_context(tc.tile_pool(name="consts", bufs=1))
        outp = es.enter_context(tc.tile_pool(name="outp", bufs=12))
        bigs = es.enter_context(tc.tile_pool(name="bigs", bufs=2))
        meds = es.enter_context(tc.tile_pool(name="meds", bufs=3))
        cols = es.enter_context(tc.tile_pool(name="cols", bufs=20))
        # PSUM: 8 banks = big 1 + acc 4 (cq / E-transpose) + small 2 + u 1.
        ps_big = es.enter_context(tc.tile_pool(name="ps_big", bufs=1, space="PSUM"))
        ps_acc = es.enter_context(tc.tile_pool(name="ps_acc", bufs=4, space="PSUM"))
        ps_sm = es.enter_context(tc.tile_pool(name="ps_sm", bufs=2, space="PSUM"))
        ps_u = es.enter_context(tc.tile_pool(name="ps_u", bufs=1, space="PSUM"))

        # ---- constants ----
        ident = consts.tile([128, 128], f32)
        masks.make_identity(nc, ident[:])
        w1_col = consts.tile([128, 1], f32)
        w2_col = consts.tile([128, 1], f32)
        w3_col = consts.tile([128, 1], f32)
        w3d = w_d.rearrange("(k d) -> k d ()", k=3)
        nc.sync.dma_start(out=w1_col[:], in_=w3d[0])
        nc.sync.dma_start(out=w2_col[:], in_=w3d[1])
        nc.sync.dma_start(out=w3_col[:], in_=w3d[2])
        ones_col = consts.tile([128, 1], f32)
        nc.vector.memset(ones_col[:], 1.0)
        ones_row = consts.tile([1, 128], f32)
        nc.vector.memset(ones_row[:], 1.0)
        f32r = mybir.dt.float32r

        msk3 = msk_d.rearrange("b q -> b q ()")
        ctx_v = ctx_d.rearrange("b (g j p) d -> b g p j d", g=N_G, p=128)
        out_v = out_d.rearrange("b (g j p) f -> b g p j f", g=N_G, p=128)

        for b in range(B_LOCAL):
            # ---------- loads: ctx into the 4 assembly tiles ----------
            gts = []
            for g in range(N_G):
                gt = outp.tile([128, 4 * 512], f32, tag="out")
                gv = gt.rearrange("p (j f) -> p j f", j=4)
                nc.sync.dma_start(out=gv[:, :, 0:128], in_=ctx_v[b, g])
                gts.append(gt)

            def ctx_blk(i):
                return gts[i // 4][:, (i % 4) * 512 : (i % 4) * 512 + 128]

            def o_blk(i, k):
                j = i % 4
                return gts[i // 4][:, j * 512 + k * 128 : j * 512 + (k + 1) * 128]

            rhs_aug = meds.tile([128, 129], f32, tag="rhs_aug")
            nc.sync.dma_start(out=rhs_aug[:, 0:128], in_=qry_d[b])
            nc.vector.memset(rhs_aug[:, 128:129], 1.0)

            mask_col_i = meds.tile([128, 1], mybir.dt.int32, tag="mask_i")
            nc.sync.dma_start(out=mask_col_i[:], in_=msk3[b])

            # ---------- query transpose + small vectors ----------
            qT_ps = ps_sm.tile([128, 128], f32, tag="small")
            nc.tensor.transpose(qT_ps[:], rhs_aug[:, 0:128], ident[:])
            qT = meds.tile([128, 128], f32, tag="qT")
            nc.scalar.copy(qT[:], qT_ps[:])

            # qw3T[d, q] = qT * w3[d]  (f32r: feeds the full-rate S^T matmul)
            qw3T = meds.tile([128, 128], f32r, tag="qw3T")
            nc.vector.tensor_scalar_mul(qw3T[:], qT[:], w3_col[:])

            # mask add column: (maskf - 1) * 1e9;  beta' = beta + maskadd
            madd_col = meds.tile([128, 1], f32, tag="madd_c")
            nc.vector.tensor_copy(madd_col[:], mask_col_i[:])  # int -> float cast
            nc.vector.tensor_scalar(
                madd_col[:], madd_col[:], 1.0, 1.0e9, op0=Alu.subtract, op1=Alu.mult
            )
            bcol_ps = ps_sm.tile([128, 1], f32, tag="small")
            nc.tensor.matmul(bcol_ps[:], qT[:], w2_col[:], start=True, stop=True)
            beta_col = meds.tile([128, 1], f32, tag="beta_c")
            nc.vector.tensor_add(beta_col[:], madd_col[:], bcol_ps[:])

            # ---------- context transpose: ctxT[d, c] ----------
            ctxT = bigs.tile([128, C], f32r, tag="ctxT")
            for g in range(4):  # groups of 4 c-tiles -> one [128,512] psum
                tr_ps = ps_big.tile([128, 512], f32, tag="big")
                for j in range(4):
                    nc.tensor.transpose(
                        tr_ps[:, j * 128 : (j + 1) * 128], ctx_blk(g * 4 + j), ident[:]
                    )
                nc.scalar.copy(ctxT[:, g * 512 : (g + 1) * 512], tr_ps[:])

            # ---------- E^T = exp(G^T + beta'[q]) in [q, c] layout ----------
            e_t = bigs.tile([128, C], f32, tag="et")
            for g in range(4):
                st_ps = ps_big.tile([128, 512], f32, tag="big")
                # f32r: full-rate single-pass fp32 matmul (vs 4 cyc/row for
                # two-pass fp32) -- N=512 keeps it off the slow path.
                nc.tensor.matmul(
                    st_ps[:],
                    qw3T[:],
                    ctxT[:, g * 512 : (g + 1) * 512],
                    start=True,
                    stop=True,
                )
                nc.scalar.activation(
                    out=e_t[:, g * 512 : (g + 1) * 512],
                    in_=st_ps[:],
                    func=AFT.Exp,
                    bias=beta_col[:],
                    scale=1.0,
                )

            # ---------- alpha[c] = ctx @ w1, one column per c-tile ----------
            alpha_ps = ps_sm.tile([128, N_CT], f32, tag="small")
            for i in range(N_CT):
                nc.tensor.matmul(
                    alpha_ps[:, i : i + 1],
                    ctxT[:, i * 128 : (i + 1) * 128].bitcast(f32),
                    w1_col[:],
                    start=True,
                    stop=True,
                )
            e_alpha = meds.tile([128, N_CT], f32, tag="e_alpha")
            nc.scalar.activation(out=e_alpha[:], in_=alpha_ps[:], func=AFT.Exp)

            # ---------- c2q & row-max of E, e_m/u chain interleaved ----------
            # u[d] = sum_c e_m[c] * ctx[c, d] accumulates tile-by-tile inside
            # this loop (its PSUM bank holds the only chain), so the q2c tail
            # after the loop is just zb/bc/out4/stores.
            e_m = meds.tile([128, N_CT], f32, tag="e_m")
            u_ps = ps_u.tile([1, 128], f32, tag="u")
            for i in range(N_CT):
                et_sl = e_t[:, i * 128 : (i + 1) * 128]
                cq_ps = ps_acc.tile([128, 129], f32, tag="acc")
                nc.tensor.matmul(cq_ps[:], et_sl, rhs_aug[:], start=True, stop=True)
                etr_ps = ps_acc.tile([128, 129], f32, tag="acc")
                nc.tensor.transpose(etr_ps[:, 0:128], et_sl, ident[:])
                maxE_c = cols.tile([128, 1], f32, tag="maxE_c")
                nc.vector.reduce_max(
                    out=maxE_c[:], in_=etr_ps[:, 0:128], axis=mybir.AxisListType.X
                )
                # e_m[:, i] = exp(alpha_i) * max_q E_i
                nc.vector.tensor_mul(
                    e_m[:, i : i + 1], e_alpha[:, i : i + 1], maxE_c[:]
                )
                nc.tensor.matmul(
                    u_ps[:],
                    e_m[:, i : i + 1],
                    ctx_blk(i),
                    start=(i == 0),
                    stop=(i == N_CT - 1),
                )
                rz = cols.tile([128, 1], f32, tag="rz")
                nc.vector.reciprocal(rz[:], cq_ps[:, 128:129])
                # c2q = (E @ query) / Z   (ACT: copy with per-partition scale)
                nc.scalar.activation(
                    out=o_blk(i, 1), in_=cq_ps[:, 0:128], func=AFT.Copy, scale=rz[:]
                )
                # out3 = ctx * c2q
                nc.vector.tensor_mul(o_blk(i, 2), ctx_blk(i), o_blk(i, 1))

            # ---------- q2c epilogue ----------
            zsum = meds.tile([128, 1], f32, tag="zsum")
            nc.vector.reduce_sum(out=zsum[:], in_=e_m[:], axis=mybir.AxisListType.X)
            zb_ps = ps_sm.tile([1, 1], f32, tag="small")
            nc.tensor.matmul(zb_ps[:], zsum[:], ones_col[:], start=True, stop=True)
            rzb = meds.tile([1, 1], f32, tag="rzb")
            nc.vector.reciprocal(rzb[:], zb_ps[:])
            q2c_row = meds.tile([1, 128], f32, tag="q2c_row")
            nc.vector.tensor_scalar_mul(q2c_row[:], u_ps[:], rzb[:])
            # broadcast to all partitions via K=1 matmul
            bc_ps = ps_sm.tile([128, 128], f32, tag="small")
            nc.tensor.matmul(bc_ps[:], ones_row[:], q2c_row[:], start=True, stop=True)
            q2c_sb = meds.tile([128, 128], f32, tag="q2c_sb")
            nc.scalar.copy(q2c_sb[:], bc_ps[:])

            # ---------- out4 + stores ----------
            # Columns 0..383 (ctx | c2q | ctx*c2q) ship as soon as the c2q
            # loop finishes; the out4 block follows, shortening the
            # end-of-batch q2c tail on the store path.
            for g in range(N_G):
                gv = gts[g].rearrange("p (j f) -> p j f", j=4)
                nc.sync.dma_start(out=out_v[b, g, :, :, 0:384], in_=gv[:, :, 0:384])
            for i in range(N_CT):
                eng = nc.gpsimd if i % 2 == 0 else nc.vector
                eng.tensor_mul(o_blk(i, 3), ctx_blk(i), q2c_sb[:])
            for g in range(N_G):
                gv = gts[g].rearrange("p (j f) -> p j f", j=4)
                nc.sync.dma_start(
                    out=out_v[b, g, :, :, 384:512], in_=gv[:, :, 384:512]
                )


def kernel(**inputs):
    global _compiled
    from concourse.bass_utils import run_bass_kernel_spmd

    context = np.ascontiguousarray(inputs["context"], dtype=np.float32)
    query = np.ascontiguousarray(inputs["query"], dtype=np.float32)
    w = np.ascontiguousarray(inputs["w"], dtype=np.float32)
    qmask = np.ascontiguousarray(inputs["query_mask"], dtype=np.int32)

    if _compiled is None:
        _compiled = _build()
    nc = _compiled

    core_ids = list(range(N_CORES))
    in_maps = []
    for k in core_ids:
        sl = slice(k * B_LOCAL, (k + 1) * B_LOCAL)
        in_maps.append(
            {
                "context": context[sl],
                "query": query[sl],
                "w": w,
                "query_mask": qmask[sl],
            }
        )

    res = run_bass_kernel_spmd(nc, in_maps, core_ids)
    outs = [res.results[k]["out"] for k in range(N_CORES)]
    return np.concatenate(outs, axis=0)



# revision 10
# speedup vs baseline: 1.1379x; 1.1379x over previous
# ContextQueryAttention (BiDAF-style) Trainium2 Bass/Tile kernel.
#
# Full-input contract: kernel(**inputs) takes the full arrays
#   context [32, 2048, 128] f32, query [32, 128, 128] f32,
#   w [384] f32, query_mask [32, 128] i32
# and returns out [32, 2048, 512] f32.
#
# Sharding: batch B=32 split 4-per-core across 8 NeuronCores (pure data
# parallel, no collectives).
#
# Math (per batch, C=2048, Q=128, D=128):
#   S[c,q] = ctx[c]@w1 + query[q]@w2 + (ctx[c]*w3)@query[q]
#          = alpha[c] + beta[q] + G[c,q]
#   a = softmax_q(S + maskadd);  c2q = a @ query
#   m[c] = max_q(S + maskadd);   b = softmax_c(m); q2c = b @ ctx
#   out = [ctx | c2q | ctx*c2q | ctx*q2c]
#
# Design notes (cost-model driven):
#  * alpha[c] cancels in softmax_q -> row softmax runs on T = G + beta'
#    (beta' = beta + mask_add) fused into the ACT exp bias in [q, c] layout.
#  * |S| = O(5), so exp() without max-subtraction is exact to fp32 roundoff.
#  * E^T = exp(T^T) is stored in *bf16*: the c2q matmuls and the E
#    transposes then run at 1 cyc/row on the PE (vs 4 for fp32).  rel-err
#    budget is 2e-2; bf16 E costs ~3e-4.
#  * max_q E per c-tile via PE-transpose of E^T; 4 tiles transposed into
#    one PSUM bank and reduced with a single 3D reduce_max.
#  * u = sum_c e_m[c]*ctx[c] computed transposed: stationary ctx tile,
#    moving e_m column -> N=1 matmuls (~free on PE).
#  * One SBUF assembly tile per batch [128, 16*512]; ctx is DMA'd straight
#    into its first column block; 2 stores/batch (cols 0:384 and 384:512).
#    All loads are issued before any store on the SP queue so a waiting
#    store never blocks a later batch's load.
#  * Elementwise work is spread: exp/scales on ACT, muls/reduces on DVE,
#    ctxT copies + half the out4 muls on gpsimd (Pool).
#
# PSUM (8 banks): big 2 (ctx transposes / G / small head+tail) + etr 2
# (E-transpose groups) + cq 3 (c2q results, 2 tiles packed per bank) +
# u 1 (accumulation chain owns its bank).

import numpy as np

C = 2048
Q = 128
D = 128
B_TOTAL = 32
N_CORES = 8
B_LOCAL = B_TOTAL // N_CORES  # 4
N_CT = C // 128  # 16 c-tiles per batch
N_G = 4  # groups of 4 c-tiles

_compiled = None


def _build():
    import concourse.bacc as bacc
    import concourse.tile as tile
    import concourse.mybir as mybir
    from concourse import masks

    f32 = mybir.dt.float32
    i32 = mybir.dt.int32

    nc = bacc.Bacc(
        "TRN2",
        target_bir_lowering=False,
        debug=False,
        num_devices=N_CORES,
    )

    ctx_d = nc.dram_tensor("context", [B_LOCAL, C, D], f32, kind="ExternalInput").ap()
    qry_d = nc.dram_tensor("query", [B_LOCAL, Q, D], f32, kind="ExternalInput").ap()
    w_d = nc.dram_tensor("w", [3 * D], f32, kind="ExternalInput").ap()
    msk_d = nc.dram_tensor("query_mask", [B_LOCAL, Q], i32, kind="ExternalInput").ap()
    out_d = nc.dram_tensor("out", [B_LOCAL, C, 4 * D], f32, kind="ExternalOutput").ap()

    with tile.TileContext(nc) as tc:
        _kernel_body(tc, out_d, ctx_d, qry_d, w_d, msk_d, mybir, masks)

    nc.compile()
    return nc


def _kernel_body(tc, out_d, ctx_d, qry_d, w_d, msk_d, mybir, masks):
    from contextlib import ExitStack

    nc = tc.nc
    f32 = mybir.dt.float32
    bf16 = mybir.dt.bfloat16
    f32r = mybir.dt.float32r
    i32 = mybir.dt.int32
    AFT = mybir.ActivationFunctionType
    Alu = mybir.AluOpType
    AX = mybir.AxisListType.X

    es = ExitStack()
    with es:
        # ---- pools ----
        consts = es.enter_context(tc.tile_pool(name="consts", bufs=1))
        outp = es.enter_context(tc.tile_pool(name="outp", bufs=4))
        bigs = es.enter_context(tc.tile_pool(name="bigs", bufs=2))
        meds = es.enter_context(tc.tile_pool(name="meds", bufs=2))
        cols = es.enter_context(tc.tile_pool(name="cols", bufs=8))
        ps_big = es.enter_context(tc.tile_pool(name="ps_big", bufs=2, space="PSUM"))
        ps_etr = es.enter_context(tc.tile_pool(name="ps_etr", bufs=2, space="PSUM"))
        ps_cq = es.enter_context(tc.tile_pool(name="ps_cq", bufs=3, space="PSUM"))
        ps_u = es.enter_context(tc.tile_pool(name="ps_u", bufs=1, space="PSUM"))

        # ---- constants ----
        ident = consts.tile([128, 128], f32)
        masks.make_identity(nc, ident[:])
        ident_bf = consts.tile([128, 128], bf16)
        masks.make_identity(nc, ident_bf[:])
        w1_col = consts.tile([128, 1], bf16)
        w2_col = consts.tile([128, 1], f32)
        w3_col = consts.tile([128, 1], f32)
        w3d = w_d.rearrange("(k d) -> k d ()", k=3)
        nc.gpsimd.dma_start(out=w1_col[:], in_=w3d[0])  # casting DMA
        nc.sync.dma_start(out=w2_col[:], in_=w3d[1])
        nc.sync.dma_start(out=w3_col[:], in_=w3d[2])
        ones_col = consts.tile([128, 1], f32)
        nc.vector.memset(ones_col[:], 1.0)
        ones_row_bf = consts.tile([1, 128], bf16)
        nc.vector.memset(ones_row_bf[:], 1.0)

        msk3 = msk_d.rearrange("b q -> b q ()")
        # c-tile i holds context rows c = i*128 + p
        ctx_v = ctx_d.rearrange("b (i p) d -> b p i d", p=128)
        out_v = out_d.rearrange("b (i p) f -> b p i f", p=128)

        # ---------- all loads up front (SP queue; no waits) ----------
        gts = []
        gvs = []
        qfs = []
        mks = []
        for b in range(B_LOCAL):
            qf = meds.tile([128, 128], f32, tag="qf", bufs=4)
            nc.sync.dma_start(out=qf[:], in_=qry_d[b])
            mk = cols.tile([128, 1], i32, tag="mask", bufs=4)
            nc.sync.dma_start(out=mk[:], in_=msk3[b])
            gt = outp.tile([128, N_CT * 512], f32, tag="out")
            gv = gt.rearrange("p (i f) -> p i f", i=N_CT)
            nc.sync.dma_start(out=gv[:, :, 0:128], in_=ctx_v[b])
            gts.append(gt)
            gvs.append(gv)
            qfs.append(qf)
            mks.append(mk)

        for b in range(B_LOCAL):
            gv = gvs[b]

            def ctx_blk(i):
                return gv[:, i, 0:128]

            def o_blk(i, k):
                return gv[:, i, k * 128 : (k + 1) * 128]

            # ---------- query prep ----------
            rhs_aug = meds.tile([128, 129], bf16, tag="rhs")
            nc.scalar.copy(rhs_aug[:, 0:128], qfs[b][:])  # f32 -> bf16 cast
            nc.vector.memset(rhs_aug[:, 128:129], 1.0)
            madd_col = cols.tile([128, 1], f32, tag="madd")
            nc.vector.tensor_copy(madd_col[:], mks[b][:])  # int -> float cast
            nc.vector.tensor_scalar(
                madd_col[:], madd_col[:], 1.0, 1.0e9, op0=Alu.subtract, op1=Alu.mult
            )

            # headA: qT (cols 0:128) + beta col (col 128)
            headA = ps_big.tile([128, 512], f32, tag="big")
            nc.tensor.transpose(headA[:, 0:128], qfs[b][:], ident[:])
            qT = meds.tile([128, 128], f32, tag="qT")
            nc.vector.tensor_copy(qT[:], headA[:, 0:128])
            qw3T = meds.tile([128, 128], bf16, tag="qw3T")
            nc.vector.tensor_scalar_mul(qw3T[:], qT[:], w3_col[:])

            # ---------- context transpose: ctxT[d, c] ----------
            ctxT = bigs.tile([128, C], bf16, tag="ctxT")
            for g in range(N_G):
                tr_ps = ps_big.tile([128, 512], f32, tag="big")
                for j in range(4):
                    nc.tensor.transpose(
                        tr_ps[:, j * 128 : (j + 1) * 128], ctx_blk(g * 4 + j), ident[:]
                    )
                if g % 2 == 0:
                    nc.scalar.copy(ctxT[:, g * 512 : (g + 1) * 512], tr_ps[:])
                else:
                    nc.vector.tensor_copy(ctxT[:, g * 512 : (g + 1) * 512], tr_ps[:])

            # beta' = beta + maskadd (matmul into headA col 128)
            nc.tensor.matmul(
                headA[:, 128:129], qT[:], w2_col[:], start=True, stop=True
            )
            beta_col = cols.tile([128, 1], f32, tag="beta")
            nc.vector.tensor_add(beta_col[:], madd_col[:], headA[:, 128:129])

            # tailB: alpha (cols 1:17), zb (col 0), q2c row (cols 32:160),
            # bc broadcast (cols 256:384)
            tailB = ps_big.tile([128, 512], f32, tag="big")

            # ---------- E^T = exp(G^T + beta'[q]) [q, c] bf16 ----------
            e_t = bigs.tile([128, C], bf16, tag="et")
            for g in range(N_G):
                st_ps = ps_big.tile([128, 512], f32, tag="big")
                nc.tensor.matmul(
                    st_ps[:],
                    qw3T[:],
                    ctxT[:, g * 512 : (g + 1) * 512],
                    start=True,
                    stop=True,
                )
                # interleave alpha columns to keep PE dense
                for i in range(4 * g, 4 * g + 4):
                    nc.tensor.matmul(
                        tailB[:, 1 + i : 2 + i],
                        ctxT[:, i * 128 : (i + 1) * 128],
                        w1_col[:],
                        start=True,
                        stop=True,
                    )
                nc.scalar.activation(
                    out=e_t[:, g * 512 : (g + 1) * 512],
                    in_=st_ps[:],
                    func=AFT.Exp,
                    bias=beta_col[:],
                    scale=1.0,
                )

            e_alpha = meds.tile([128, N_CT], f32, tag="ealpha")
            nc.scalar.activation(out=e_alpha[:], in_=tailB[:, 1:17], func=AFT.Exp)

            # ---------- c2q / row-max / u loop over groups ----------
            e_m = meds.tile([128, N_CT], f32, tag="em")
            uT_ps = ps_u.tile([128, 1], f32, tag="u")
            cq_slots = {}
            for g in range(N_G):
                etr = ps_etr.tile([128, 512], bf16, tag="etr")
                rzs = {}
                for j in range(4):
                    i = 4 * g + j
                    if j % 2 == 0:
                        cq2 = ps_cq.tile([128, 258], f32, tag="cq")
                    cqs = cq2[:, 129 * (j % 2) : 129 * (j % 2) + 129]
                    cq_slots[i] = cqs
                    et_sl = e_t[:, i * 128 : (i + 1) * 128]
                    nc.tensor.matmul(cqs, et_sl, rhs_aug[:], start=True, stop=True)
                    nc.tensor.transpose(
                        etr[:, j * 128 : (j + 1) * 128], et_sl, ident_bf[:]
                    )
                    if j % 2 == 1:
                        # one reciprocal per pair: Z cols sit at 128 and 257
                        rz2 = cols.tile([128, 2], f32, tag="rz")
                        zv = cq2.rearrange("p (k n) -> p k n", k=2)[:, :, 128]
                        nc.vector.reciprocal(rz2[:], zv)
                        rzs[j - 1] = rz2[:, 0:1]
                        rzs[j] = rz2[:, 1:2]
                for j in range(4):
                    i = 4 * g + j
                    # c2q = (E @ [qry|1]) / Z  (ACT copy with per-partition scale)
                    nc.scalar.activation(
                        out=o_blk(i, 1),
                        in_=cq_slots[i][:, 0:128],
                        func=AFT.Copy,
                        scale=rzs[j],
                    )
                # row max over q of the 4 transposed tiles, then e_m
                maxE = cols.tile([128, 4], f32, tag="maxE")
                nc.vector.reduce_max(
                    out=maxE[:], in_=etr.rearrange("p (j q) -> p j q", j=4), axis=AX
                )
                nc.vector.tensor_mul(
                    e_m[:, 4 * g : 4 * g + 4], e_alpha[:, 4 * g : 4 * g + 4], maxE[:]
                )
                # out3 = ctx * c2q (SBUF-only: gpsimd can help)
                for j in range(4):
                    i = 4 * g + j
                    eng = nc.gpsimd if j == 3 else nc.vector
                    eng.tensor_mul(o_blk(i, 2), ctx_blk(i), o_blk(i, 1))
                # u chain for the previous group (e_m ready by then)
                if g > 0:
                    for i in range(4 * (g - 1), 4 * g):
                        nc.tensor.matmul(
                            uT_ps[:],
                            ctx_blk(i),
                            e_m[:, i : i + 1],
                            start=(i == 0),
                            stop=False,
                        )
            for i in range(4 * (N_G - 1), N_CT):
                nc.tensor.matmul(
                    uT_ps[:],
                    ctx_blk(i),
                    e_m[:, i : i + 1],
                    start=False,
                    stop=(i == N_CT - 1),
                )

            # ---------- q2c epilogue ----------
            zsum = cols.tile([128, 1], f32, tag="zsum")
            nc.vector.reduce_sum(out=zsum[:], in_=e_m[:], axis=AX)
            nc.tensor.matmul(
                tailB[0:1, 0:1], zsum[:], ones_col[:], start=True, stop=True
            )
            u_sb = cols.tile([128, 1], f32, tag="usb")
            nc.vector.tensor_copy(u_sb[:], uT_ps[:])
            nc.tensor.transpose(tailB[0:1, 32:160], u_sb[:], ident[:])
            rzb = cols.tile([1, 1], f32, tag="rzb")
            nc.vector.reciprocal(rzb[:], tailB[0:1, 0:1])
            q2c_row = cols.tile([1, 128], bf16, tag="q2crow")
            nc.scalar.activation(
                out=q2c_row[:], in_=tailB[0:1, 32:160], func=AFT.Copy, scale=rzb[:]
            )
            nc.tensor.matmul(
                tailB[:, 256:384], ones_row_bf[:], q2c_row[:], start=True, stop=True
            )
            q2c_sb = meds.tile([128, 128], f32, tag="q2csb")
            nc.vector.tensor_copy(q2c_sb[:], tailB[:, 256:384])

            # ---------- out4 + stores ----------
            for i in range(N_CT):
                eng = nc.gpsimd if i % 8 < 5 else nc.vector
                eng.tensor_mul(o_blk(i, 3), ctx_blk(i), q2c_sb[:])
            nc.sync.dma_start(out=out_v[b][:, :, 0:384], in_=gv[:, :, 0:384])
            nc.sync.dma_start(out=out_v[b][:, :, 384:512], in_=gv[:, :, 384:512])


def kernel(**inputs):
    global _compiled
    from concourse.bass_utils import run_bass_kernel_spmd

    context = np.ascontiguousarray(inputs["context"], dtype=np.float32)
    query = np.ascontiguousarray(inputs["query"], dtype=np.float32)
    w = np.ascontiguousarray(inputs["w"], dtype=np.float32)
    qmask = np.ascontiguousarray(inputs["query_mask"], dtype=np.int32)

    if _compiled is None:
        _compiled = _build()
    nc = _compiled

    core_ids = list(range(N_CORES))
    in_maps = []
    for k in core_ids:
        sl = slice(k * B_LOCAL, (k + 1) * B_LOCAL)
        in_maps.append(
            {
                "context": context[sl],
                "query": query[sl],
                "w": w,
                "query_mask": qmask[sl],
            }
        )

    res = run_bass_kernel_spmd(nc, in_maps, core_ids)
    outs = [res.results[k]["out"] for k in range(N_CORES)]
    return np.concatenate(outs, axis=0)


# revision 13
# speedup vs baseline: 1.1914x; 1.0470x over previous
# ContextQueryAttention (BiDAF-style) Trainium2 Bass/Tile kernel.
#
# Full-input contract: kernel(**inputs) takes the full arrays
#   context [32, 2048, 128] f32, query [32, 128, 128] f32,
#   w [384] f32, query_mask [32, 128] i32
# and returns out [32, 2048, 512] f32.
#
# Sharding: batch B=32 split 4-per-core across 8 NeuronCores (pure data
# parallel, no collectives).
#
# Math (per batch, C=2048, Q=128, D=128):
#   S[c,q] = ctx[c]@w1 + query[q]@w2 + (ctx[c]*w3)@query[q]
#          = alpha[c] + beta[q] + G[c,q]
#   a = softmax_q(S + maskadd);  c2q = a @ query
#   m[c] = max_q(S + maskadd);   b = softmax_c(m); q2c = b @ ctx
#   out = [ctx | c2q | ctx*c2q | ctx*q2c]
#
# Design notes (cost-model driven):
#  * alpha[c] cancels in softmax_q -> row softmax runs on T = G + beta'
#    (beta' = beta + mask_add) fused into the ACT exp bias in [q, c] layout.
#  * |S| = O(5), so exp() without max-subtraction is exact to fp32 roundoff.
#  * E^T = exp(T^T) is stored in *bf16*: the c2q matmuls and the E
#    transposes then run at 1 cyc/row on the PE (vs 4 for fp32).  rel-err
#    budget is 2e-2; bf16 E costs ~3e-4.
#  * max_q E per c-tile via PE-transpose of E^T; 4 tiles transposed into
#    one PSUM bank and reduced with a single 3D reduce_max.
#  * u = sum_c e_m[c]*ctx[c] computed transposed: stationary ctx tile,
#    moving e_m column -> N=1 matmuls (~free on PE).
#  * One SBUF assembly tile per batch [128, 16*512]; ctx is DMA'd straight
#    into its first column block; 2 stores/batch (cols 0:384 and 384:512).
#    All loads are issued before any store on the SP queue so a waiting
#    store never blocks a later batch's load.
#  * Elementwise work is spread: exp/scales on ACT, muls/reduces on DVE,
#    ctxT copies + half the out4 muls on gpsimd (Pool).
#
# PSUM (8 banks): big 2 (ctx transposes / G / small head+tail) + etr 2
# (E-transpose groups) + cq 3 (c2q results, 2 tiles packed per bank) +
# u 1 (accumulation chain owns its bank).

import numpy as np

C = 2048
Q = 128
D = 128
B_TOTAL = 32
N_CORES = 8
B_LOCAL = B_TOTAL // N_CORES  # 4
N_CT = C // 128  # 16 c-tiles per batch
N_G = 4  # groups of 4 c-tiles

_compiled = None


def _build():
    import concourse.bacc as bacc
    import concourse.tile as tile
    import concourse.mybir as mybir
    from concourse import masks

    f32 = mybir.dt.float32
    i32 = mybir.dt.int32

    nc = bacc.Bacc(
        "TRN2",
        target_bir_lowering=False,
        debug=False,
        num_devices=N_CORES,
    )

    ctx_d = nc.dram_tensor("context", [B_LOCAL, C, D], f32, kind="ExternalInput").ap()
    qry_d = nc.dram_tensor("query", [B_LOCAL, Q, D], f32, kind="ExternalInput").ap()
    w_d = nc.dram_tensor("w", [3 * D], f32, kind="ExternalInput").ap()
    msk_d = nc.dram_tensor("query_mask", [B_LOCAL, Q], i32, kind="ExternalInput").ap()
    out_d = nc.dram_tensor("out", [B_LOCAL, C, 4 * D], f32, kind="ExternalOutput").ap()

    with tile.TileContext(nc) as tc:
        _kernel_body(tc, out_d, ctx_d, qry_d, w_d, msk_d, mybir, masks)

    nc.compile()
    return nc


def _kernel_body(tc, out_d, ctx_d, qry_d, w_d, msk_d, mybir, masks):
    from contextlib import ExitStack

    nc = tc.nc
    f32 = mybir.dt.float32
    bf16 = mybir.dt.bfloat16
    f32r = mybir.dt.float32r
    i32 = mybir.dt.int32
    AFT = mybir.ActivationFunctionType
    Alu = mybir.AluOpType
    AX = mybir.AxisListType.X

    es = ExitStack()
    with es:
        # ---- pools ----
        consts = es.enter_context(tc.tile_pool(name="consts", bufs=1))
        outp = es.enter_context(tc.tile_pool(name="outp", bufs=4))
        bigs = es.enter_context(tc.tile_pool(name="bigs", bufs=2))
        meds = es.enter_context(tc.tile_pool(name="meds", bufs=2))
        cols = es.enter_context(tc.tile_pool(name="cols", bufs=8))
        ps_big = es.enter_context(tc.tile_pool(name="ps_big", bufs=2, space="PSUM"))
        ps_etr = es.enter_context(tc.tile_pool(name="ps_etr", bufs=2, space="PSUM"))
        ps_cq = es.enter_context(tc.tile_pool(name="ps_cq", bufs=3, space="PSUM"))
        ps_u = es.enter_context(tc.tile_pool(name="ps_u", bufs=1, space="PSUM"))

        # ---- constants ----
        ident = consts.tile([128, 128], f32)
        masks.make_identity(nc, ident[:])
        ident_bf = consts.tile([128, 128], bf16)
        masks.make_identity(nc, ident_bf[:])
        w1_col = consts.tile([128, 1], bf16)
        w2_col = consts.tile([128, 1], f32)
        w3_col = consts.tile([128, 1], f32)
        w3d = w_d.rearrange("(k d) -> k d ()", k=3)
        nc.gpsimd.dma_start(out=w1_col[:], in_=w3d[0])  # casting DMA
        nc.sync.dma_start(out=w2_col[:], in_=w3d[1])
        nc.sync.dma_start(out=w3_col[:], in_=w3d[2])
        ones_col = consts.tile([128, 1], f32)
        nc.vector.memset(ones_col[:], 1.0)
        ones_row_bf = consts.tile([1, 128], bf16)
        nc.vector.memset(ones_row_bf[:], 1.0)

        msk3 = msk_d.rearrange("b q -> b q ()")
        # c-tile i holds context rows c = i*128 + p
        ctx_v = ctx_d.rearrange("b (i p) d -> b p i d", p=128)
        out_v = out_d.rearrange("b (i p) f -> b p i f", p=128)

        # ---------- loads (SP queue) ----------
        # Small loads + the first two batches' ctx go up front (no waits);
        # ctx for batches 2/3 is deferred into the store stream so an
        # early-batch store never queues behind a load it doesn't need.
        gts = []
        gvs = []
        qfs = []
        mks = []
        for b in range(B_LOCAL):
            qf = meds.tile([128, 128], f32, tag="qf", bufs=4)
            nc.sync.dma_start(out=qf[:], in_=qry_d[b])
            mk = cols.tile([128, 1], i32, tag="mask", bufs=4)
            nc.sync.dma_start(out=mk[:], in_=msk3[b])
            gt = outp.tile([128, N_CT * 512], f32, tag="out")
            gv = gt.rearrange("p (i f) -> p i f", i=N_CT)
            gts.append(gt)
            gvs.append(gv)
            qfs.append(qf)
            mks.append(mk)

        def load_ctx(b, quarters=True):
            # quartered: group g of 4 c-tiles arrives separately so batch-0
            # transposes can start after the first 256 KiB.
            if quarters:
                for g in range(N_G):
                    nc.sync.dma_start(
                        out=gvs[b][:, 4 * g : 4 * g + 4, 0:128],
                        in_=ctx_v[b][:, 4 * g : 4 * g + 4, :],
                    )
            else:
                nc.sync.dma_start(out=gvs[b][:, :, 0:128], in_=ctx_v[b])

        load_ctx(0, quarters=True)
        load_ctx(1, quarters=True)

        for b in range(B_LOCAL):
            gv = gvs[b]

            def ctx_blk(i):
                return gv[:, i, 0:128]

            def o_blk(i, k):
                return gv[:, i, k * 128 : (k + 1) * 128]

            # ---------- query prep ----------
            rhs_aug = meds.tile([128, 129], bf16, tag="rhs")
            nc.scalar.copy(rhs_aug[:, 0:128], qfs[b][:])  # f32 -> bf16 cast
            nc.vector.memset(rhs_aug[:, 128:129], 1.0)
            madd_col = cols.tile([128, 1], f32, tag="madd")
            nc.vector.tensor_copy(madd_col[:], mks[b][:])  # int -> float cast
            nc.vector.tensor_scalar(
                madd_col[:], madd_col[:], 1.0, 1.0e9, op0=Alu.subtract, op1=Alu.mult
            )

            # headA: qT (cols 0:128) + beta col (col 128)
            headA = ps_big.tile([128, 512], f32, tag="big")
            nc.tensor.transpose(headA[:, 0:128], qfs[b][:], ident[:])
            qT = meds.tile([128, 128], f32, tag="qT")
            nc.vector.tensor_copy(qT[:], headA[:, 0:128])
            qw3T = meds.tile([128, 128], bf16, tag="qw3T")
            nc.vector.tensor_scalar_mul(qw3T[:], qT[:], w3_col[:])

            # ---------- context transpose: ctxT[d, c] ----------
            ctxT = bigs.tile([128, C], bf16, tag="ctxT")
            for g in range(N_G):
                tr_ps = ps_big.tile([128, 512], f32, tag="big")
                for j in range(4):
                    nc.tensor.transpose(
                        tr_ps[:, j * 128 : (j + 1) * 128], ctx_blk(g * 4 + j), ident[:]
                    )
                if g % 2 == 0:
                    nc.scalar.copy(ctxT[:, g * 512 : (g + 1) * 512], tr_ps[:])
                else:
                    nc.vector.tensor_copy(ctxT[:, g * 512 : (g + 1) * 512], tr_ps[:])

            # beta' = beta + maskadd (matmul into headA col 128)
            nc.tensor.matmul(
                headA[:, 128:129], qT[:], w2_col[:], start=True, stop=True
            )
            beta_col = cols.tile([128, 1], f32, tag="beta")
            nc.vector.tensor_add(beta_col[:], madd_col[:], headA[:, 128:129])

            # tailB: alpha (cols 1:17), zb (col 0), q2c row (cols 32:160),
            # bc broadcast (cols 256:384)
            tailB = ps_big.tile([128, 512], f32, tag="big")

            # ---------- E^T = exp(G^T + beta'[q]) [q, c] bf16 ----------
            e_t = bigs.tile([128, C], bf16, tag="et")
            for g in range(N_G):
                st_ps = ps_big.tile([128, 512], f32, tag="big")
                nc.tensor.matmul(
                    st_ps[:],
                    qw3T[:],
                    ctxT[:, g * 512 : (g + 1) * 512],
                    start=True,
                    stop=True,
                )
                # interleave alpha columns to keep PE dense
                for i in range(4 * g, 4 * g + 4):
                    nc.tensor.matmul(
                        tailB[:, 1 + i : 2 + i],
                        ctxT[:, i * 128 : (i + 1) * 128],
                        w1_col[:],
                        start=True,
                        stop=True,
                    )
                nc.scalar.activation(
                    out=e_t[:, g * 512 : (g + 1) * 512],
                    in_=st_ps[:],
                    func=AFT.Exp,
                    bias=beta_col[:],
                    scale=1.0,
                )

            e_alpha = meds.tile([128, N_CT], f32, tag="ealpha")
            nc.scalar.activation(out=e_alpha[:], in_=tailB[:, 1:17], func=AFT.Exp)

            # ---------- c2q / row-max / u loop over groups ----------
            e_m = meds.tile([128, N_CT], f32, tag="em")
            uT_ps = ps_u.tile([128, 1], f32, tag="u")
            cq_slots = {}
            for g in range(N_G):
                etr = ps_etr.tile([128, 512], bf16, tag="etr")
                rzs = {}
                for j in range(4):
                    i = 4 * g + j
                    if j % 2 == 0:
                        cq2 = ps_cq.tile([128, 258], f32, tag="cq")
                    cqs = cq2[:, 129 * (j % 2) : 129 * (j % 2) + 129]
                    cq_slots[i] = cqs
                    et_sl = e_t[:, i * 128 : (i + 1) * 128]
                    nc.tensor.matmul(cqs, et_sl, rhs_aug[:], start=True, stop=True)
                    nc.tensor.transpose(
                        etr[:, j * 128 : (j + 1) * 128], et_sl, ident_bf[:]
                    )
                    if j % 2 == 1:
                        # one reciprocal per pair: Z cols sit at 128 and 257
                        rz2 = cols.tile([128, 2], f32, tag="rz")
                        zv = cq2.rearrange("p (k n) -> p k n", k=2)[:, :, 128]
                        nc.vector.reciprocal(rz2[:], zv)
                        rzs[j - 1] = rz2[:, 0:1]
                        rzs[j] = rz2[:, 1:2]
                for j in range(4):
                    i = 4 * g + j
                    # c2q = (E @ [qry|1]) / Z  (ACT copy with per-partition scale)
                    nc.scalar.activation(
                        out=o_blk(i, 1),
                        in_=cq_slots[i][:, 0:128],
                        func=AFT.Copy,
                        scale=rzs[j],
                    )
                # row max over q of the 4 transposed tiles, then e_m
                maxE = cols.tile([128, 4], f32, tag="maxE")
                nc.vector.reduce_max(
                    out=maxE[:], in_=etr.rearrange("p (j q) -> p j q", j=4), axis=AX
                )
                nc.vector.tensor_mul(
                    e_m[:, 4 * g : 4 * g + 4], e_alpha[:, 4 * g : 4 * g + 4], maxE[:]
                )
                # out3 = ctx * c2q (SBUF-only: gpsimd can help)
                for j in range(4):
                    i = 4 * g + j
                    eng = nc.gpsimd if j == 3 else nc.vector
                    eng.tensor_mul(o_blk(i, 2), ctx_blk(i), o_blk(i, 1))
                # this group's [ctx | c2q | ctx*c2q] is final: ship it
                nc.sync.dma_start(
                    out=out_v[b][:, 4 * g : 4 * g + 4, 0:384],
                    in_=gv[:, 4 * g : 4 * g + 4, 0:384],
                )
                # u chain for the previous group (e_m ready by then)
                if g > 0:
                    for i in range(4 * (g - 1), 4 * g):
                        nc.tensor.matmul(
                            uT_ps[:],
                            ctx_blk(i),
                            e_m[:, i : i + 1],
                            start=(i == 0),
                            stop=False,
                        )
            for i in range(4 * (N_G - 1), N_CT):
                nc.tensor.matmul(
                    uT_ps[:],
                    ctx_blk(i),
                    e_m[:, i : i + 1],
                    start=False,
                    stop=(i == N_CT - 1),
                )

            # ---------- q2c epilogue ----------
            zsum = cols.tile([128, 1], f32, tag="zsum")
            nc.vector.reduce_sum(out=zsum[:], in_=e_m[:], axis=AX)
            nc.tensor.matmul(
                tailB[0:1, 0:1], zsum[:], ones_col[:], start=True, stop=True
            )
            u_sb = cols.tile([128, 1], f32, tag="usb")
            nc.vector.tensor_copy(u_sb[:], uT_ps[:])
            nc.tensor.transpose(tailB[0:1, 32:160], u_sb[:], ident[:])
            rzb = cols.tile([1, 1], f32, tag="rzb")
            nc.vector.reciprocal(rzb[:], tailB[0:1, 0:1])
            q2c_row = cols.tile([1, 128], bf16, tag="q2crow")
            nc.scalar.activation(
                out=q2c_row[:], in_=tailB[0:1, 32:160], func=AFT.Copy, scale=rzb[:]
            )
            nc.tensor.matmul(
                tailB[:, 256:384], ones_row_bf[:], q2c_row[:], start=True, stop=True
            )
            q2c_sb = meds.tile([128, 128], f32, tag="q2csb")
            nc.vector.tensor_copy(q2c_sb[:], tailB[:, 256:384])

            # ---------- out4 + remaining stores ----------
            for i in range(8):
                eng = nc.gpsimd if i % 8 < 5 else nc.vector
                eng.tensor_mul(o_blk(i, 3), ctx_blk(i), q2c_sb[:])
            nc.sync.dma_start(
                out=out_v[b][:, 0:8, 384:512], in_=gv[:, 0:8, 384:512]
            )
            for i in range(8, N_CT):
                eng = nc.gpsimd if i % 8 < 5 else nc.vector
                eng.tensor_mul(o_blk(i, 3), ctx_blk(i), q2c_sb[:])
            nc.sync.dma_start(
                out=out_v[b][:, 8:N_CT, 384:512], in_=gv[:, 8:N_CT, 384:512]
            )
            # deferred ctx loads for batches 2/3 ride behind this batch's
            # stores (their data isn't needed until then)
            if b + 2 < B_LOCAL:
                load_ctx(b + 2, quarters=False)


def kernel(**inputs):
    global _compiled
    from concourse.bass_utils import run_bass_kernel_spmd

    context = np.ascontiguousarray(inputs["context"], dtype=np.float32)
    query = np.ascontiguousarray(inputs["query"], dtype=np.float32)
    w = np.ascontiguousarray(inputs["w"], dtype=np.float32)
    qmask = np.ascontiguousarray(inputs["query_mask"], dtype=np.int32)

    if _compiled is None:
        _compiled = _build()
    nc = _compiled

    core_ids = list(range(N_CORES))
    in_maps = []
    for k in core_ids:
        sl = slice(k * B_LOCAL, (k + 1) * B_LOCAL)
        in_maps.append(
            {
                "context": context[sl],
                "query": query[sl],
                "w": w,
                "query_mask": qmask[sl],
            }
        )

    res = run_bass_kernel_spmd(nc, in_maps, core_ids)
    outs = [res.results[k]["out"] for k in range(N_CORES)]
    return np.concatenate(outs, axis=0)


# revision 17
# speedup vs baseline: 1.2234x; 1.0269x over previous
# ContextQueryAttention (BiDAF-style) Trainium2 Bass/Tile kernel.
#
# Full-input contract: kernel(**inputs) takes the full arrays
#   context [32, 2048, 128] f32, query [32, 128, 128] f32,
#   w [384] f32, query_mask [32, 128] i32
# and returns out [32, 2048, 512] f32.
#
# Sharding: batch B=32 split 4-per-core across 8 NeuronCores (pure data
# parallel, no collectives).
#
# Math (per batch, C=2048, Q=128, D=128):
#   S[c,q] = ctx[c]@w1 + query[q]@w2 + (ctx[c]*w3)@query[q]
#          = alpha[c] + beta[q] + G[c,q]
#   a = softmax_q(S + maskadd);  c2q = a @ query
#   m[c] = max_q(S + maskadd);   b = softmax_c(m); q2c = b @ ctx
#   out = [ctx | c2q | ctx*c2q | ctx*q2c]
#
# Design notes (cost-model driven):
#  * alpha[c] cancels in softmax_q -> row softmax runs on T = G + beta'
#    (beta' = beta + mask_add) fused into the ACT exp bias in [q, c] layout.
#  * |S| = O(5), so exp() without max-subtraction is exact to fp32 roundoff.
#  * E^T = exp(T^T) is stored in *bf16*: the c2q matmuls and the E
#    transposes then run at 1 cyc/row on the PE (vs 4 for fp32).  rel-err
#    budget is 2e-2; bf16 E costs ~3e-4.
#  * max_q E per c-tile via PE-transpose of E^T; 4 tiles transposed into
#    one PSUM bank and reduced with a single 3D reduce_max.
#  * u = sum_c e_m[c]*ctx[c] computed transposed: stationary ctx tile,
#    moving e_m column -> N=1 matmuls (~free on PE).
#  * One SBUF assembly tile per batch [128, 16*512]; ctx is DMA'd straight
#    into its first column block; 2 stores/batch (cols 0:384 and 384:512).
#    All loads are issued before any store on the SP queue so a waiting
#    store never blocks a later batch's load.
#  * Elementwise work is spread: exp/scales on ACT, muls/reduces on DVE,
#    ctxT copies + half the out4 muls on gpsimd (Pool).
#
# PSUM (8 banks): big 2 (ctx transposes / G / small head+tail) + etr 2
# (E-transpose groups) + cq 3 (c2q results, 2 tiles packed per bank) +
# u 1 (accumulation chain owns its bank).

import numpy as np

C = 2048
Q = 128
D = 128
B_TOTAL = 32
N_CORES = 8
B_LOCAL = B_TOTAL // N_CORES  # 4
N_CT = C // 128  # 16 c-tiles per batch
N_G = 4  # groups of 4 c-tiles

_compiled = None


def _build():
    import concourse.bacc as bacc
    import concourse.tile as tile
    import concourse.mybir as mybir
    from concourse import masks

    f32 = mybir.dt.float32
    i32 = mybir.dt.int32

    nc = bacc.Bacc(
        "TRN2",
        target_bir_lowering=False,
        debug=False,
        num_devices=N_CORES,
    )

    ctx_d = nc.dram_tensor("context", [B_LOCAL, C, D], f32, kind="ExternalInput").ap()
    qry_d = nc.dram_tensor("query", [B_LOCAL, Q, D], f32, kind="ExternalInput").ap()
    w_d = nc.dram_tensor("w", [3 * D], f32, kind="ExternalInput").ap()
    msk_d = nc.dram_tensor("query_mask", [B_LOCAL, Q], i32, kind="ExternalInput").ap()
    out_d = nc.dram_tensor("out", [B_LOCAL, C, 4 * D], f32, kind="ExternalOutput").ap()

    with tile.TileContext(nc) as tc:
        _kernel_body(tc, out_d, ctx_d, qry_d, w_d, msk_d, mybir, masks)

    nc.compile()
    return nc


def _kernel_body(tc, out_d, ctx_d, qry_d, w_d, msk_d, mybir, masks):
    from contextlib import ExitStack

    nc = tc.nc
    f32 = mybir.dt.float32
    bf16 = mybir.dt.bfloat16
    f32r = mybir.dt.float32r
    i32 = mybir.dt.int32
    AFT = mybir.ActivationFunctionType
    Alu = mybir.AluOpType
    AX = mybir.AxisListType.X

    es = ExitStack()
    with es:
        # ---- pools ----
        consts = es.enter_context(tc.tile_pool(name="consts", bufs=1))
        outp = es.enter_context(tc.tile_pool(name="outp", bufs=4))
        bigs = es.enter_context(tc.tile_pool(name="bigs", bufs=2))
        meds = es.enter_context(tc.tile_pool(name="meds", bufs=2))
        cols = es.enter_context(tc.tile_pool(name="cols", bufs=8))
        ps_big = es.enter_context(tc.tile_pool(name="ps_big", bufs=2, space="PSUM"))
        ps_etr = es.enter_context(tc.tile_pool(name="ps_etr", bufs=2, space="PSUM"))
        ps_cq = es.enter_context(tc.tile_pool(name="ps_cq", bufs=3, space="PSUM"))
        ps_u = es.enter_context(tc.tile_pool(name="ps_u", bufs=1, space="PSUM"))

        # ---- constants ----
        ident = consts.tile([128, 128], f32)
        masks.make_identity(nc, ident[:])
        ident_bf = consts.tile([128, 128], bf16)
        masks.make_identity(nc, ident_bf[:])
        # w loads ride the gpsimd SWDGE queue: no HWDGE contention with the
        # ctx/qry loads on SP at program start.
        w1_col = consts.tile([128, 1], bf16)
        w2_col = consts.tile([128, 1], f32)
        w3_col = consts.tile([128, 1], f32)
        w3d = w_d.rearrange("(k d) -> k d ()", k=3)
        nc.gpsimd.dma_start(out=w1_col[:], in_=w3d[0])  # casting DMA
        nc.gpsimd.dma_start(out=w2_col[:], in_=w3d[1])
        nc.gpsimd.dma_start(out=w3_col[:], in_=w3d[2])
        ones_col = consts.tile([128, 1], f32)
        nc.vector.memset(ones_col[:], 1.0)
        ones_row_bf = consts.tile([1, 128], bf16)
        nc.vector.memset(ones_row_bf[:], 1.0)

        msk3 = msk_d.rearrange("b q -> b q ()")
        # c-tile i holds context rows c = i*128 + p
        ctx_v = ctx_d.rearrange("b (i p) d -> b p i d", p=128)
        out_v = out_d.rearrange("b (i p) f -> b p i f", p=128)

        # ---------- loads (SP queue) ----------
        # Small loads + the first two batches' ctx go up front (no waits);
        # ctx for batches 2/3 is deferred into the store stream so an
        # early-batch store never queues behind a load it doesn't need.
        gts = []
        gvs = []
        qfs = []
        mks = []
        for b in range(B_LOCAL):
            qf = meds.tile([128, 128], f32, tag="qf", bufs=4)
            mk = cols.tile([128, 1], i32, tag="mask", bufs=4)
            gt = outp.tile([128, N_CT * 512], f32, tag="out")
            gv = gt.rearrange("p (i f) -> p i f", i=N_CT)
            gts.append(gt)
            gvs.append(gv)
            qfs.append(qf)
            mks.append(mk)

        def load_qm(b):
            nc.sync.dma_start(out=qfs[b][:], in_=qry_d[b])
            nc.sync.dma_start(out=mks[b][:], in_=msk3[b])

        def load_ctx(b, quarters=True):
            # quartered: group g of 4 c-tiles arrives separately so batch-0
            # transposes can start after the first 256 KiB.
            if quarters:
                for g in range(N_G):
                    nc.sync.dma_start(
                        out=gvs[b][:, 4 * g : 4 * g + 4, 0:128],
                        in_=ctx_v[b][:, 4 * g : 4 * g + 4, :],
                    )
            else:
                nc.sync.dma_start(out=gvs[b][:, :, 0:128], in_=ctx_v[b])

        nc.sync.dma_start(
            out=gvs[0][:, 0:4, 0:128], in_=ctx_v[0][:, 0:4, :]
        )
        load_qm(0)
        for g in range(1, N_G):
            nc.sync.dma_start(
                out=gvs[0][:, 4 * g : 4 * g + 4, 0:128],
                in_=ctx_v[0][:, 4 * g : 4 * g + 4, :],
            )
        load_ctx(1, quarters=False)
        load_qm(1)

        for b in range(B_LOCAL):
            gv = gvs[b]

            def ctx_blk(i):
                return gv[:, i, 0:128]

            def o_blk(i, k):
                return gv[:, i, k * 128 : (k + 1) * 128]

            # ---------- query prep ----------
            rhs_aug = meds.tile([128, 129], bf16, tag="rhs")
            nc.scalar.copy(rhs_aug[:, 0:128], qfs[b][:])  # f32 -> bf16 cast
            nc.vector.memset(rhs_aug[:, 128:129], 1.0)
            madd_col = cols.tile([128, 1], f32, tag="madd")
            nc.vector.tensor_copy(madd_col[:], mks[b][:])  # int -> float cast
            nc.vector.tensor_scalar(
                madd_col[:], madd_col[:], 1.0, 1.0e9, op0=Alu.subtract, op1=Alu.mult
            )

            # headA: qT (cols 0:128) + beta col (col 128)
            headA = ps_big.tile([128, 512], f32, tag="big")
            nc.tensor.transpose(headA[:, 0:128], qfs[b][:], ident[:])
            qT = meds.tile([128, 128], f32, tag="qT")
            nc.vector.tensor_copy(qT[:], headA[:, 0:128])
            qw3T = meds.tile([128, 128], bf16, tag="qw3T")
            nc.vector.tensor_scalar_mul(qw3T[:], qT[:], w3_col[:])

            # ---------- context transpose: ctxT[d, c] ----------
            ctxT = bigs.tile([128, C], bf16, tag="ctxT")
            for g in range(N_G):
                tr_ps = ps_big.tile([128, 512], f32, tag="big")
                for j in range(4):
                    nc.tensor.transpose(
                        tr_ps[:, j * 128 : (j + 1) * 128], ctx_blk(g * 4 + j), ident[:]
                    )
                if g % 2 == 0:
                    nc.scalar.copy(ctxT[:, g * 512 : (g + 1) * 512], tr_ps[:])
                else:
                    nc.vector.tensor_copy(ctxT[:, g * 512 : (g + 1) * 512], tr_ps[:])

            # beta' = beta + maskadd (matmul into headA col 128)
            nc.tensor.matmul(
                headA[:, 128:129], qT[:], w2_col[:], start=True, stop=True
            )
            beta_col = cols.tile([128, 1], f32, tag="beta")
            nc.vector.tensor_add(beta_col[:], madd_col[:], headA[:, 128:129])

            # tailB: alpha (cols 1:17), zb (col 0), q2c row (cols 32:160),
            # bc broadcast (cols 256:384)
            tailB = ps_big.tile([128, 512], f32, tag="big")

            # ---------- E^T = exp(G^T + beta'[q]) [q, c] bf16 ----------
            e_t = bigs.tile([128, C], bf16, tag="et")
            for g in range(N_G):
                st_ps = ps_big.tile([128, 512], f32, tag="big")
                nc.tensor.matmul(
                    st_ps[:],
                    qw3T[:],
                    ctxT[:, g * 512 : (g + 1) * 512],
                    start=True,
                    stop=True,
                )
                # interleave alpha columns to keep PE dense
                for i in range(4 * g, 4 * g + 4):
                    nc.tensor.matmul(
                        tailB[:, 1 + i : 2 + i],
                        ctxT[:, i * 128 : (i + 1) * 128],
                        w1_col[:],
                        start=True,
                        stop=True,
                    )
                nc.scalar.activation(
                    out=e_t[:, g * 512 : (g + 1) * 512],
                    in_=st_ps[:],
                    func=AFT.Exp,
                    bias=beta_col[:],
                    scale=1.0,
                )

            e_alpha = meds.tile([128, N_CT], f32, tag="ealpha")
            nc.scalar.activation(out=e_alpha[:], in_=tailB[:, 1:17], func=AFT.Exp)

            # ---------- c2q / row-max / u loop over groups ----------
            e_m = meds.tile([128, N_CT], f32, tag="em")
            uT_ps = ps_u.tile([128, 1], f32, tag="u")
            cq_slots = {}
            for g in range(N_G):
                etr = ps_etr.tile([128, 512], bf16, tag="etr")
                rzs = {}
                for j in range(4):
                    i = 4 * g + j
                    if j % 2 == 0:
                        cq2 = ps_cq.tile([128, 258], f32, tag="cq")
                    cqs = cq2[:, 129 * (j % 2) : 129 * (j % 2) + 129]
                    cq_slots[i] = cqs
                    et_sl = e_t[:, i * 128 : (i + 1) * 128]
                    nc.tensor.matmul(cqs, et_sl, rhs_aug[:], start=True, stop=True)
                    nc.tensor.transpose(
                        etr[:, j * 128 : (j + 1) * 128], et_sl, ident_bf[:]
                    )
                    if j % 2 == 1:
                        # one reciprocal per pair: Z cols sit at 128 and 257
                        rz2 = cols.tile([128, 2], f32, tag="rz")
                        zv = cq2.rearrange("p (k n) -> p k n", k=2)[:, :, 128]
                        nc.vector.reciprocal(rz2[:], zv)
                        rzs[j - 1] = rz2[:, 0:1]
                        rzs[j] = rz2[:, 1:2]
                for j in range(4):
                    i = 4 * g + j
                    # c2q = (E @ [qry|1]) / Z  (ACT copy with per-partition scale)
                    nc.scalar.activation(
                        out=o_blk(i, 1),
                        in_=cq_slots[i][:, 0:128],
                        func=AFT.Copy,
                        scale=rzs[j],
                    )
                # row max over q of the 4 transposed tiles, then e_m
                maxE = cols.tile([128, 4], f32, tag="maxE")
                nc.vector.reduce_max(
                    out=maxE[:], in_=etr.rearrange("p (j q) -> p j q", j=4), axis=AX
                )
                nc.vector.tensor_mul(
                    e_m[:, 4 * g : 4 * g + 4], e_alpha[:, 4 * g : 4 * g + 4], maxE[:]
                )
                # out3 = ctx * c2q (SBUF-only: gpsimd can help)
                for j in range(4):
                    i = 4 * g + j
                    eng = nc.gpsimd if j == 3 else nc.vector
                    eng.tensor_mul(o_blk(i, 2), ctx_blk(i), o_blk(i, 1))
                # this group's [ctx | c2q | ctx*c2q] is final: ship it
                nc.sync.dma_start(
                    out=out_v[b][:, 4 * g : 4 * g + 4, 0:384],
                    in_=gv[:, 4 * g : 4 * g + 4, 0:384],
                )
                # deferred small loads hide their HWDGE under store transfers
                if b == 0 and g == 0:
                    load_qm(2)
                if b == 0 and g == 1:
                    load_qm(3)
                # u chain for the previous group (e_m ready by then)
                if g > 0:
                    for i in range(4 * (g - 1), 4 * g):
                        nc.tensor.matmul(
                            uT_ps[:],
                            ctx_blk(i),
                            e_m[:, i : i + 1],
                            start=(i == 0),
                            stop=False,
                        )
            for i in range(4 * (N_G - 1), N_CT):
                nc.tensor.matmul(
                    uT_ps[:],
                    ctx_blk(i),
                    e_m[:, i : i + 1],
                    start=False,
                    stop=(i == N_CT - 1),
                )

            # ---------- q2c epilogue ----------
            zsum = cols.tile([128, 1], f32, tag="zsum")
            nc.vector.reduce_sum(out=zsum[:], in_=e_m[:], axis=AX)
            nc.tensor.matmul(
                tailB[0:1, 0:1], zsum[:], ones_col[:], start=True, stop=True
            )
            u_sb = cols.tile([128, 1], f32, tag="usb")
            nc.vector.tensor_copy(u_sb[:], uT_ps[:])
            nc.tensor.transpose(tailB[0:1, 32:160], u_sb[:], ident[:])
            rzb = cols.tile([1, 1], f32, tag="rzb")
            nc.vector.reciprocal(rzb[:], tailB[0:1, 0:1])
            q2c_row = cols.tile([1, 128], bf16, tag="q2crow")
            nc.scalar.activation(
                out=q2c_row[:], in_=tailB[0:1, 32:160], func=AFT.Copy, scale=rzb[:]
            )
            nc.tensor.matmul(
                tailB[:, 256:384], ones_row_bf[:], q2c_row[:], start=True, stop=True
            )
            q2c_sb = meds.tile([128, 128], f32, tag="q2csb")
            nc.vector.tensor_copy(q2c_sb[:], tailB[:, 256:384])

            # ---------- out4 + remaining stores (4 pieces) ----------
            # 3 DVE + 1 gpsimd mul per piece; each piece ships on completion
            for piece in range(4):
                for j in range(4):
                    i = 4 * piece + j
                    eng = nc.gpsimd if j == 1 else nc.vector
                    eng.tensor_mul(o_blk(i, 3), ctx_blk(i), q2c_sb[:])
                nc.sync.dma_start(
                    out=out_v[b][:, 4 * piece : 4 * piece + 4, 384:512],
                    in_=gv[:, 4 * piece : 4 * piece + 4, 384:512],
                )
            # deferred ctx loads for batches 2/3 ride behind this batch's
            # stores (their data isn't needed until then)
            if b + 2 < B_LOCAL:
                load_ctx(b + 2, quarters=False)


def kernel(**inputs):
    global _compiled
    from concourse.bass_utils import run_bass_kernel_spmd

    context = np.ascontiguousarray(inputs["context"], dtype=np.float32)
    query = np.ascontiguousarray(inputs["query"], dtype=np.float32)
    w = np.ascontiguousarray(inputs["w"], dtype=np.float32)
    qmask = np.ascontiguousarray(inputs["query_mask"], dtype=np.int32)

    if _compiled is None:
        _compiled = _build()
    nc = _compiled

    core_ids = list(range(N_CORES))
    in_maps = []
    for k in core_ids:
        sl = slice(k * B_LOCAL, (k + 1) * B_LOCAL)
        in_maps.append(
            {
                "context": context[sl],
                "query": query[sl],
                "w": w,
                "query_mask": qmask[sl],
            }
        )

    res = run_bass_kernel_spmd(nc, in_maps, core_ids)
    outs = [res.results[k]["out"] for k in range(N_CORES)]
    return np.concatenate(outs, axis=0)


# revision 20
# speedup vs baseline: 1.2387x; 1.0125x over previous
# ContextQueryAttention (BiDAF-style) Trainium2 Bass/Tile kernel.
#
# Full-input contract: kernel(**inputs) takes the full arrays
#   context [32, 2048, 128] f32, query [32, 128, 128] f32,
#   w [384] f32, query_mask [32, 128] i32
# and returns out [32, 2048, 512] f32.
#
# Sharding: batch B=32 split 4-per-core across 8 NeuronCores (pure data
# parallel, no collectives).
#
# Math (per batch, C=2048, Q=128, D=128):
#   S[c,q] = ctx[c]@w1 + query[q]@w2 + (ctx[c]*w3)@query[q]
#          = alpha[c] + beta[q] + G[c,q]
#   a = softmax_q(S + maskadd);  c2q = a @ query
#   m[c] = max_q(S + maskadd);   b = softmax_c(m); q2c = b @ ctx
#   out = [ctx | c2q | ctx*c2q | ctx*q2c]
#
# Design notes (cost-model driven):
#  * alpha[c] cancels in softmax_q -> row softmax runs on T = G + beta'
#    (beta' = beta + mask_add) fused into the ACT exp bias in [q, c] layout.
#  * |S| = O(5), so exp() without max-subtraction is exact to fp32 roundoff.
#  * E^T = exp(T^T) is stored in *bf16*: the c2q matmuls and the E
#    transposes then run at 1 cyc/row on the PE (vs 4 for fp32).  rel-err
#    budget is 2e-2; bf16 E costs ~3e-4.
#  * max_q E per c-tile via PE-transpose of E^T; 4 tiles transposed into
#    one PSUM bank and reduced with a single 3D reduce_max.
#  * u = sum_c e_m[c]*ctx[c] computed transposed: stationary ctx tile,
#    moving e_m column -> N=1 matmuls (~free on PE).
#  * One SBUF assembly tile per batch [128, 16*512]; ctx is DMA'd straight
#    into its first column block; 2 stores/batch (cols 0:384 and 384:512).
#    All loads are issued before any store on the SP queue so a waiting
#    store never blocks a later batch's load.
#  * Elementwise work is spread: exp/scales on ACT, muls/reduces on DVE,
#    ctxT copies + half the out4 muls on gpsimd (Pool).
#
# PSUM (8 banks): big 2 (ctx transposes / G / small head+tail) + etr 2
# (E-transpose groups) + cq 3 (c2q results, 2 tiles packed per bank) +
# u 1 (accumulation chain owns its bank).

import numpy as np

C = 2048
Q = 128
D = 128
B_TOTAL = 32
N_CORES = 8
B_LOCAL = B_TOTAL // N_CORES  # 4
N_CT = C // 128  # 16 c-tiles per batch
N_G = 4  # groups of 4 c-tiles

_compiled = None


def _build():
    import concourse.bacc as bacc
    import concourse.tile as tile
    import concourse.mybir as mybir
    from concourse import masks

    f32 = mybir.dt.float32
    i32 = mybir.dt.int32

    nc = bacc.Bacc(
        "TRN2",
        target_bir_lowering=False,
        debug=False,
        num_devices=N_CORES,
    )

    ctx_d = nc.dram_tensor("context", [B_LOCAL, C, D], f32, kind="ExternalInput").ap()
    qry_d = nc.dram_tensor("query", [B_LOCAL, Q, D], f32, kind="ExternalInput").ap()
    w_d = nc.dram_tensor("w", [3 * D], f32, kind="ExternalInput").ap()
    msk_d = nc.dram_tensor("query_mask", [B_LOCAL, Q], i32, kind="ExternalInput").ap()
    out_d = nc.dram_tensor("out", [B_LOCAL, C, 4 * D], f32, kind="ExternalOutput").ap()

    with tile.TileContext(nc) as tc:
        _kernel_body(tc, out_d, ctx_d, qry_d, w_d, msk_d, mybir, masks)

    nc.compile()
    return nc


def _kernel_body(tc, out_d, ctx_d, qry_d, w_d, msk_d, mybir, masks):
    from contextlib import ExitStack

    nc = tc.nc
    f32 = mybir.dt.float32
    bf16 = mybir.dt.bfloat16
    f32r = mybir.dt.float32r
    i32 = mybir.dt.int32
    AFT = mybir.ActivationFunctionType
    Alu = mybir.AluOpType
    AX = mybir.AxisListType.X

    es = ExitStack()
    with es:
        # ---- pools ----
        consts = es.enter_context(tc.tile_pool(name="consts", bufs=1))
        outp = es.enter_context(tc.tile_pool(name="outp", bufs=4))
        bigs = es.enter_context(tc.tile_pool(name="bigs", bufs=2))
        meds = es.enter_context(tc.tile_pool(name="meds", bufs=2))
        cols = es.enter_context(tc.tile_pool(name="cols", bufs=8))
        ps_big = es.enter_context(tc.tile_pool(name="ps_big", bufs=2, space="PSUM"))
        ps_etr = es.enter_context(tc.tile_pool(name="ps_etr", bufs=2, space="PSUM"))
        ps_cq = es.enter_context(tc.tile_pool(name="ps_cq", bufs=3, space="PSUM"))
        ps_u = es.enter_context(tc.tile_pool(name="ps_u", bufs=1, space="PSUM"))

        # ---- constants ----
        ident = consts.tile([128, 128], f32)
        masks.make_identity(nc, ident[:])
        ident_bf = consts.tile([128, 128], bf16)
        masks.make_identity(nc, ident_bf[:])
        # w loads ride the gpsimd SWDGE queue: no HWDGE contention with the
        # ctx/qry loads on SP at program start.
        w1_col = consts.tile([128, 1], bf16)
        w2_col = consts.tile([128, 1], f32)
        w3_col = consts.tile([128, 1], f32)
        w3d = w_d.rearrange("(k d) -> k d ()", k=3)
        nc.gpsimd.dma_start(out=w1_col[:], in_=w3d[0])  # casting DMA
        nc.gpsimd.dma_start(out=w2_col[:], in_=w3d[1])
        nc.gpsimd.dma_start(out=w3_col[:], in_=w3d[2])
        ones_col = consts.tile([128, 1], f32)
        nc.vector.memset(ones_col[:], 1.0)
        ones_row_bf = consts.tile([1, 128], bf16)
        nc.vector.memset(ones_row_bf[:], 1.0)

        msk3 = msk_d.rearrange("b q -> b q ()")
        # c-tile i holds context rows c = i*128 + p
        ctx_v = ctx_d.rearrange("b (i p) d -> b p i d", p=128)
        out_v = out_d.rearrange("b (i p) f -> b p i f", p=128)

        # ---------- loads (SP queue) ----------
        # Small loads + the first two batches' ctx go up front (no waits);
        # ctx for batches 2/3 is deferred into the store stream so an
        # early-batch store never queues behind a load it doesn't need.
        gts = []
        gvs = []
        qfs = []
        mks = []
        for b in range(B_LOCAL):
            qf = meds.tile([128, 128], f32, tag="qf", bufs=4)
            mk = cols.tile([128, 1], i32, tag="mask", bufs=4)
            gt = outp.tile([128, N_CT * 512], f32, tag="out")
            gv = gt.rearrange("p (i f) -> p i f", i=N_CT)
            gts.append(gt)
            gvs.append(gv)
            qfs.append(qf)
            mks.append(mk)

        def load_qm(b):
            nc.sync.dma_start(out=qfs[b][:], in_=qry_d[b])
            nc.sync.dma_start(out=mks[b][:], in_=msk3[b])

        def load_ctx(b, quarters=True):
            # quartered: group g of 4 c-tiles arrives separately so batch-0
            # transposes can start after the first 256 KiB.
            if quarters:
                for g in range(N_G):
                    nc.sync.dma_start(
                        out=gvs[b][:, 4 * g : 4 * g + 4, 0:128],
                        in_=ctx_v[b][:, 4 * g : 4 * g + 4, :],
                    )
            else:
                nc.sync.dma_start(out=gvs[b][:, :, 0:128], in_=ctx_v[b])

        nc.sync.dma_start(
            out=gvs[0][:, 0:4, 0:128], in_=ctx_v[0][:, 0:4, :]
        )
        load_qm(0)
        for g in range(1, N_G):
            nc.sync.dma_start(
                out=gvs[0][:, 4 * g : 4 * g + 4, 0:128],
                in_=ctx_v[0][:, 4 * g : 4 * g + 4, :],
            )
        load_ctx(1, quarters=False)
        load_qm(1)

        for b in range(B_LOCAL):
            gv = gvs[b]

            def ctx_blk(i):
                return gv[:, i, 0:128]

            def o_blk(i, k):
                return gv[:, i, k * 128 : (k + 1) * 128]

            # ---------- query prep ----------
            rhs_aug = meds.tile([128, 129], bf16, tag="rhs")
            nc.vector.tensor_copy(rhs_aug[:, 0:128], qfs[b][:])  # f32 -> bf16
            nc.vector.memset(rhs_aug[:, 128:129], 1.0)
            madd_col = cols.tile([128, 1], f32, tag="madd")
            nc.vector.tensor_copy(madd_col[:], mks[b][:])  # int -> float cast
            nc.vector.tensor_scalar(
                madd_col[:], madd_col[:], 1.0, 1.0e9, op0=Alu.subtract, op1=Alu.mult
            )

            # headA: qT (cols 0:128) + beta col (col 128)
            headA = ps_big.tile([128, 512], f32, tag="big")
            nc.tensor.transpose(headA[:, 0:128], qfs[b][:], ident[:])
            qT = meds.tile([128, 128], f32, tag="qT")
            nc.vector.tensor_copy(qT[:], headA[:, 0:128])
            qw3T = meds.tile([128, 128], bf16, tag="qw3T")
            nc.vector.tensor_scalar_mul(qw3T[:], qT[:], w3_col[:])

            # ---------- context transpose: ctxT[d, c] ----------
            ctxT = bigs.tile([128, C], bf16, tag="ctxT")
            for g in range(N_G):
                tr_ps = ps_big.tile([128, 512], f32, tag="big")
                for j in range(4):
                    nc.tensor.transpose(
                        tr_ps[:, j * 128 : (j + 1) * 128], ctx_blk(g * 4 + j), ident[:]
                    )
                if g % 2 == 0:
                    nc.scalar.copy(ctxT[:, g * 512 : (g + 1) * 512], tr_ps[:])
                else:
                    nc.vector.tensor_copy(ctxT[:, g * 512 : (g + 1) * 512], tr_ps[:])

            # beta' = beta + maskadd (matmul into headA col 128)
            nc.tensor.matmul(
                headA[:, 128:129], qT[:], w2_col[:], start=True, stop=True
            )
            beta_col = cols.tile([128, 1], f32, tag="beta")
            nc.vector.tensor_add(beta_col[:], madd_col[:], headA[:, 128:129])

            # tailB: alpha (cols 1:17), zb (col 0), q2c row (cols 32:160),
            # bc broadcast (cols 256:384)
            tailB = ps_big.tile([128, 512], f32, tag="big")

            # ---------- E^T = exp(G^T + beta'[q]) [q, c] bf16 ----------
            e_t = bigs.tile([128, C], bf16, tag="et")
            for g in range(N_G):
                st_ps = ps_big.tile([128, 512], f32, tag="big")
                nc.tensor.matmul(
                    st_ps[:],
                    qw3T[:],
                    ctxT[:, g * 512 : (g + 1) * 512],
                    start=True,
                    stop=True,
                )
                # interleave alpha columns to keep PE dense
                for i in range(4 * g, 4 * g + 4):
                    nc.tensor.matmul(
                        tailB[:, 1 + i : 2 + i],
                        ctxT[:, i * 128 : (i + 1) * 128],
                        w1_col[:],
                        start=True,
                        stop=True,
                    )
                nc.scalar.activation(
                    out=e_t[:, g * 512 : (g + 1) * 512],
                    in_=st_ps[:],
                    func=AFT.Exp,
                    bias=beta_col[:],
                    scale=1.0,
                )

            e_alpha = meds.tile([128, N_CT], f32, tag="ealpha")
            nc.scalar.activation(out=e_alpha[:], in_=tailB[:, 1:17], func=AFT.Exp)

            # ---------- c2q / row-max / u loop over groups ----------
            e_m = meds.tile([128, N_CT], f32, tag="em")
            uT_ps = ps_u.tile([128, 1], f32, tag="u")
            cq_slots = {}
            for g in range(N_G):
                etr = ps_etr.tile([128, 512], bf16, tag="etr")
                rzs = {}
                for j in range(4):
                    i = 4 * g + j
                    if j % 2 == 0:
                        cq2 = ps_cq.tile([128, 258], f32, tag="cq")
                    cqs = cq2[:, 129 * (j % 2) : 129 * (j % 2) + 129]
                    cq_slots[i] = cqs
                    et_sl = e_t[:, i * 128 : (i + 1) * 128]
                    nc.tensor.matmul(cqs, et_sl, rhs_aug[:], start=True, stop=True)
                    nc.tensor.transpose(
                        etr[:, j * 128 : (j + 1) * 128], et_sl, ident_bf[:]
                    )
                    if j % 2 == 1:
                        # one reciprocal per pair: Z cols sit at 128 and 257
                        rz2 = cols.tile([128, 2], f32, tag="rz")
                        zv = cq2.rearrange("p (k n) -> p k n", k=2)[:, :, 128]
                        nc.vector.reciprocal(rz2[:], zv)
                        rzs[j - 1] = rz2[:, 0:1]
                        rzs[j] = rz2[:, 1:2]
                for j in range(4):
                    i = 4 * g + j
                    # c2q = (E @ [qry|1]) / Z  (ACT copy with per-partition scale)
                    nc.scalar.activation(
                        out=o_blk(i, 1),
                        in_=cq_slots[i][:, 0:128],
                        func=AFT.Copy,
                        scale=rzs[j],
                    )
                # row max over q of the 4 transposed tiles, then e_m
                maxE = cols.tile([128, 4], f32, tag="maxE")
                nc.vector.reduce_max(
                    out=maxE[:], in_=etr.rearrange("p (j q) -> p j q", j=4), axis=AX
                )
                nc.vector.tensor_mul(
                    e_m[:, 4 * g : 4 * g + 4], e_alpha[:, 4 * g : 4 * g + 4], maxE[:]
                )
                # out3 = ctx * c2q (SBUF-only: gpsimd can help)
                for j in range(4):
                    i = 4 * g + j
                    eng = nc.gpsimd if j >= 2 else nc.vector
                    eng.tensor_mul(o_blk(i, 2), ctx_blk(i), o_blk(i, 1))
                # this group's [ctx | c2q | ctx*c2q] is final: ship it.
                # g3 is held back (b<3) to bridge the next batch's prologue
                # drought at the DMA queue.
                if g < 3 or b == B_LOCAL - 1:
                    nc.sync.dma_start(
                        out=out_v[b][:, 4 * g : 4 * g + 4, 0:384],
                        in_=gv[:, 4 * g : 4 * g + 4, 0:384],
                    )
                # deferred small loads hide their HWDGE under store transfers
                if b == 0 and g == 0:
                    load_qm(2)
                if b == 0 and g == 1:
                    load_qm(3)
                # u chain for the previous group (e_m ready by then)
                if g > 0:
                    for i in range(4 * (g - 1), 4 * g):
                        nc.tensor.matmul(
                            uT_ps[:],
                            ctx_blk(i),
                            e_m[:, i : i + 1],
                            start=(i == 0),
                            stop=False,
                        )
            for i in range(4 * (N_G - 1), N_CT):
                nc.tensor.matmul(
                    uT_ps[:],
                    ctx_blk(i),
                    e_m[:, i : i + 1],
                    start=False,
                    stop=(i == N_CT - 1),
                )

            # ---------- q2c epilogue ----------
            zsum = cols.tile([128, 1], f32, tag="zsum")
            nc.vector.reduce_sum(out=zsum[:], in_=e_m[:], axis=AX)
            nc.tensor.matmul(
                tailB[0:1, 0:1], zsum[:], ones_col[:], start=True, stop=True
            )
            u_sb = cols.tile([128, 1], f32, tag="usb")
            nc.vector.tensor_copy(u_sb[:], uT_ps[:])
            nc.tensor.transpose(tailB[0:1, 32:160], u_sb[:], ident[:])
            rzb = cols.tile([1, 1], f32, tag="rzb")
            nc.vector.reciprocal(rzb[:], tailB[0:1, 0:1])
            q2c_row = cols.tile([1, 128], bf16, tag="q2crow")
            nc.scalar.activation(
                out=q2c_row[:], in_=tailB[0:1, 32:160], func=AFT.Copy, scale=rzb[:]
            )
            nc.tensor.matmul(
                tailB[:, 256:384], ones_row_bf[:], q2c_row[:], start=True, stop=True
            )
            q2c_sb = meds.tile([128, 128], f32, tag="q2csb")
            nc.vector.tensor_copy(q2c_sb[:], tailB[:, 256:384])

            # ---------- out4 + remaining stores (4 pieces) ----------
            # 3 DVE + 1 gpsimd mul per piece; each piece ships on completion
            for piece in range(4):
                for j in range(4):
                    i = 4 * piece + j
                    eng = nc.gpsimd if j == 1 else nc.vector
                    eng.tensor_mul(o_blk(i, 3), ctx_blk(i), q2c_sb[:])
                nc.sync.dma_start(
                    out=out_v[b][:, 4 * piece : 4 * piece + 4, 384:512],
                    in_=gv[:, 4 * piece : 4 * piece + 4, 384:512],
                )
                if piece == 1 and b < B_LOCAL - 1:
                    # the held-back g3 column store rides here: ready long
                    # ago, it keeps DMA fed through the next prologue
                    nc.sync.dma_start(
                        out=out_v[b][:, 12:16, 0:384], in_=gv[:, 12:16, 0:384]
                    )
            # deferred ctx loads for batches 2/3 ride behind this batch's
            # stores (their data isn't needed until then)
            if b + 2 < B_LOCAL:
                load_ctx(b + 2, quarters=False)


def kernel(**inputs):
    global _compiled
    from concourse.bass_utils import run_bass_kernel_spmd

    context = np.ascontiguousarray(inputs["context"], dtype=np.float32)
    query = np.ascontiguousarray(inputs["query"], dtype=np.float32)
    w = np.ascontiguousarray(inputs["w"], dtype=np.float32)
    qmask = np.ascontiguousarray(inputs["query_mask"], dtype=np.int32)

    if _compiled is None:
        _compiled = _build()
    nc = _compiled

    core_ids = list(range(N_CORES))
    in_maps = []
    for k in core_ids:
        sl = slice(k * B_LOCAL, (k + 1) * B_LOCAL)
        in_maps.append(
            {
                "context": context[sl],
                "query": query[sl],
                "w": w,
                "query_mask": qmask[sl],
            }
        )

    res = run_bass_kernel_spmd(nc, in_maps, core_ids)
    outs = [res.results[k]["out"] for k in range(N_CORES)]
    return np.concatenate(outs, axis=0)
